# revision 2
# baseline (speedup 1.0000x reference)
"""Trainium2 Bass kernel for nn_Net_19387482374339.

Net: per-batch-element scalar LSTM (IN=1, HID=1) over SEQ=3 steps, then a
Linear(18 -> 1) over flattened groups of 6 consecutive batch elements.

Strategy (v2):
  - Pure data parallel over 8 NeuronCores (batch split).
  - x is cast to fp16 on host and uploaded in a partition-major layout
    [T, NP=126, SEQ*F] so each tile is ONE contiguous 2.25MB DMA.
    126 partitions = 21 output groups x 6 members, so the final linear
    becomes 3 tiny TensorE matmuls (contract over partitions) into PSUM.
  - ACT-op minimization (ScalarE is the wall at 1 elem/cyc/lane):
    t=0 is collapsed into a single shared tanh basis: tau = tanh(al*x0+be),
    with h1 ~= a*tau + d and c1 ~= a2*tau + d2 (both 1-D functions of x0,
    fitted at build time; affine parts folded into downstream scales/biases
    and the linear weights). Steps t=1,2 are exact. 11 ACT evals/element
    (vs 14 naive), each using ACT's free input affine (scale,bias).
  - DVE-op minimization: each gate pre-activation is ONE fused
    scalar_tensor_tensor; the cell state is kept in a 1/a2-scaled basis so
    the c-update is 3 ops with no extra rescales. 16 DVE ops/element.
  - Cell-state c in a scaled basis c_hat = c/a2 so no extra DVE rescales;
    tanh(c) reads c_hat with ACT input scale=a2.
  - PSUM->SBUF output copies run on ACT, placed at the two points in the
    pipeline where ACT would otherwise stall on a DVE dependency chain.
  - Software pipeline: tile k's heavy stages interleave with k+1's combines
    and k+2's DMA/tau so every engine's in-order stream stays fed.
"""

import numpy as np

N_CORES = 8
B = 12582912
SEQ = 3
Bc = B // N_CORES            # 1,572,864 elements per core
GC = Bc // 6                 # 262,144 output groups per core
NP = 126                     # SBUF partitions used (21 groups of 6)
NQ = 21                      # group blocks
T = 7                        # tiles per core
F = 1792                     # elements per partition per tile
PAD_E = T * NP * F           # 1,580,544 padded elements per core
CHUNKS = [(c0, min(512, F - c0)) for c0 in range(0, F, 512)]

_CACHE = {}


def _fit_shared_tanh(wi, wf, wg, wo, bi, bg, bo):
    """Fit h1(x) ~= a*tanh(al*x+be)+d and c1(x) ~= a2*tanh(al*x+be)+d2
    (shared inner argument) over x~N(0,1). Returns params + gauss-rms errs.
    Pure numpy (no scipy)."""
    xs = np.linspace(-6.2, 6.2, 2401)
    gw = np.exp(-xs * xs / 2)
    wts = gw + 3e-4
    sig = lambda z: 1.0 / (1.0 + np.exp(-z))
    i0 = sig(wi * xs + bi)
    g0 = np.tanh(wg * xs + bg)
    o0 = sig(wo * xs + bo)
    c1x = i0 * g0
    h1x = o0 * np.tanh(c1x)

    sw = np.sum(wts)
    swy_h = np.sum(wts * h1x)
    swy_c = np.sum(wts * c1x)

    def cost(al, be):
        tau = np.tanh(al * xs + be)
        swt = wts * tau
        s_tt = np.sum(swt * tau)
        s_t = np.sum(swt)
        det = s_tt * sw - s_t * s_t
        if abs(det) < 1e-12:
            return np.inf, None
        tot = 0.0
        prm = []
        for y, swy in ((h1x, swy_h), (c1x, swy_c)):
            sty = np.sum(swt * y)
            a = (sty * sw - s_t * swy) / det
            d = (s_tt * swy - s_t * sty) / det
            r = a * tau + d - y
            tot += np.sum(wts * r * r)
            prm.append((a, d))
        return tot, prm

    best = (np.inf, None, None, None)
    for al in np.linspace(0.05, 1.5, 59):
        for be in np.linspace(-2.5, 2.5, 51):
            c, prm = cost(al, be)
            if c < best[0]:
                best = (c, al, be, prm)
    span_al, span_be = 0.06, 0.12
    for _ in range(6):
        _, al0, be0, _ = best
        for al in np.linspace(al0 - span_al, al0 + span_al, 13):
            for be in np.linspace(be0 - span_be, be0 + span_be, 13):
                c, prm = cost(al, be)
                if c < best[0]:
                    best = (c, al, be, prm)
        span_al /= 4.0
        span_be /= 4.0
    _, al, be, ((a, d), (a2, d2)) = best
    tau = np.tanh(al * xs + be)
    rms_h = np.sqrt(np.average((a * tau + d - h1x) ** 2, weights=gw))
    rms_c = np.sqrt(np.average((a2 * tau + d2 - c1x) ** 2, weights=gw))
    return al, be, a, d, a2, d2, rms_h, rms_c


def _build_kernel(key):
    (wi, wf, wg, wo, ui, uf, ug, uo, bi, bf, bg, bo,
     fit_ok, al, be, a, d, a2, d2) = key
    import concourse.bacc as bacc
    import concourse.tile as tile
    from concourse import mybir

    dt = mybir.dt
    AF = mybir.ActivationFunctionType
    ALU = mybir.AluOpType
    F16 = dt.float16

    gates = ((wi, ui, bi, AF.Sigmoid),
             (wf, uf, bf, AF.Sigmoid),
             (wg, ug, bg, AF.Tanh),
             (wo, uo, bo, AF.Sigmoid))

    inv_a2 = 1.0 / a2
    delta = d2 / a2

    # ACT bias constants must pre-exist as const APs.
    biases = set()
    biases.add(float(be))
    for (w_, u_, b_, _fn) in gates:
        biases.add(float(b_ + u_ * d))   # t=1 folded bias
        biases.add(float(b_))            # t=2 bias
    if not fit_ok:
        biases.update((float(bi), float(bg), float(bo)))

    nc = bacc.Bacc("TRN2", target_bir_lowering=False, debug=False)
    for v in sorted(biases):
        if v == 0.0:
            continue
        t = nc.alloc_sbuf_tensor(f"const-user-{v!r}", [128, 1], dt.float32)
        nc.gpsimd.memset(t.ap(), v)
        nc.const_aps.aps[(dt.float32, v)] = t.ap()
    nc.all_engine_barrier()

    xd = nc.declare_dram_parameter("x", [T, NP, SEQ * F], F16, isOutput=False)
    wds = [nc.declare_dram_parameter(f"w{t + 1}", [NP, NQ], F16, isOutput=False)
           for t in range(3)]
    outd = nc.declare_dram_parameter("out", [T, NQ, F], F16, isOutput=True)

    with tile.TileContext(nc) as tc:
        with tc.tile_pool(name="wpool", bufs=1) as wpool, \
             tc.tile_pool(name="sbuf", bufs=2) as pool, \
             tc.tile_pool(name="psum", bufs=2, space="PSUM") as psum_pool:
            wt = []
            for wd in wds:
                w = wpool.tile([NP, NQ], F16, tag=f"w{wd.name}")
                nc.sync.dma_start(w[:], wd[:])
                wt.append(w)

            st = [dict() for _ in range(T)]  # per-tile tensors

            def DMA(k):
                xt = pool.tile([NP, SEQ * F], F16, tag="x", bufs=3, name=f"x_{k}")
                nc.sync.dma_start(xt[:], xd[k])
                st[k]["x"] = xt

            def TAU(k):
                x0 = st[k]["x"][:, 0:F]
                tau = pool.tile([NP, F], F16, tag="tau", bufs=3, name=f"tau_{k}")
                if fit_ok:
                    nc.scalar.activation(tau[:], x0, AF.Tanh,
                                         bias=float(be), scale=float(al))
                    st[k]["tau"] = tau
                    st[k]["c1"] = tau   # c1 read via (tau + delta) * a2
                else:
                    # exact t0: tau := h1, plus separate c1 tensor
                    i0 = pool.tile([NP, F], F16, tag="gi", bufs=3, name=f"i0_{k}")
                    g0 = pool.tile([NP, F], F16, tag="gg", bufs=3, name=f"g0_{k}")
                    o0 = pool.tile([NP, F], F16, tag="go", bufs=3, name=f"o0_{k}")
                    nc.scalar.activation(i0[:], x0, AF.Sigmoid, bias=float(bi), scale=float(wi))
                    nc.scalar.activation(g0[:], x0, AF.Tanh, bias=float(bg), scale=float(wg))
                    nc.scalar.activation(o0[:], x0, AF.Sigmoid, bias=float(bo), scale=float(wo))
                    c1 = pool.tile([NP, F], F16, tag="c1", bufs=3, name=f"c1_{k}")
                    nc.vector.tensor_tensor(c1[:], i0[:], g0[:], ALU.mult)
                    tc1 = pool.tile([NP, F], F16, tag="tc", bufs=2, name=f"tc1_{k}")
                    nc.scalar.activation(tc1[:], c1[:], AF.Tanh, bias=0.0, scale=1.0)
                    nc.vector.tensor_tensor(tau[:], o0[:], tc1[:], ALU.mult)
                    st[k]["tau"] = tau
                    st[k]["c1"] = c1

            def S1(k):
                x1 = st[k]["x"][:, F:2 * F]
                tau = st[k]["tau"]
                ss = []
                for gi_, (w_, u_, b_, fn) in enumerate(gates):
                    s = pool.tile([NP, F], F16, tag=f"s{gi_}", bufs=3, name=f"s1{gi_}_{k}")
                    nc.vector.scalar_tensor_tensor(
                        s[:], x1, float(w_ / (u_ * a)), tau[:], ALU.mult, ALU.add)
                    ss.append(s)
                st[k]["s1"] = ss

            def G1(k):
                gs = []
                for gi_, (w_, u_, b_, fn) in enumerate(gates):
                    gt = pool.tile([NP, F], F16, tag=f"g{gi_}", bufs=3, name=f"g1{gi_}_{k}")
                    nc.scalar.activation(gt[:], st[k]["s1"][gi_][:], fn,
                                         bias=float(b_ + u_ * d), scale=float(u_ * a))
                    gs.append(gt)
                st[k]["g1"] = gs

            def CH2(k):
                i1, f1, g1, o1 = st[k]["g1"]
                m1 = pool.tile([NP, F], F16, tag="tmA", bufs=2, name=f"m1_{k}")
                nc.vector.scalar_tensor_tensor(m1[:], i1[:], float(inv_a2), g1[:],
                                               ALU.mult, ALU.mult)
                R = pool.tile([NP, F], F16, tag="tmB", bufs=2, name=f"R_{k}")
                if fit_ok:
                    nc.vector.scalar_tensor_tensor(R[:], st[k]["c1"][:], float(delta),
                                                   f1[:], ALU.add, ALU.mult)
                else:
                    nc.vector.tensor_tensor(R[:], st[k]["c1"][:], f1[:], ALU.mult)
                ch2 = pool.tile([NP, F], F16, tag="ch", bufs=3, name=f"ch2_{k}")
                nc.vector.tensor_tensor(ch2[:], m1[:], R[:], ALU.add)
                st[k]["ch2"] = ch2

            def TC2(k):
                tc2 = pool.tile([NP, F], F16, tag="tc", bufs=2, name=f"tc2_{k}")
                nc.scalar.activation(tc2[:], st[k]["ch2"][:], AF.Tanh,
                                     bias=0.0, scale=float(a2))
                st[k]["tc2"] = tc2

            def H2(k):
                h2 = pool.tile([NP, F], F16, tag="h2", bufs=2, name=f"h2_{k}")
                nc.vector.tensor_tensor(h2[:], st[k]["g1"][3][:], st[k]["tc2"][:], ALU.mult)
                st[k]["h2"] = h2

            def S2(k):
                x2 = st[k]["x"][:, 2 * F:3 * F]
                h2 = st[k]["h2"]
                ss = []
                for gi_, (w_, u_, b_, fn) in enumerate(gates):
                    s = pool.tile([NP, F], F16, tag=f"s{gi_}", bufs=3, name=f"s2{gi_}_{k}")
                    nc.vector.scalar_tensor_tensor(
                        s[:], x2, float(w_ / u_), h2[:], ALU.mult, ALU.add)
                    ss.append(s)
                st[k]["s2"] = ss

            def G2(k, rng):
                gs = st[k].setdefault("g2", [None] * 4)
                for gi_ in rng:
                    (w_, u_, b_, fn) = gates[gi_]
                    gt = pool.tile([NP, F], F16, tag=f"g{gi_}", bufs=3, name=f"g2{gi_}_{k}")
                    nc.scalar.activation(gt[:], st[k]["s2"][gi_][:], fn,
                                         bias=float(b_), scale=float(u_))
                    gs[gi_] = gt

            def CH3(k):
                i2, f2, g2, _o2 = st[k]["g2"]
                m2 = pool.tile([NP, F], F16, tag="tmA", bufs=2, name=f"m2_{k}")
                nc.vector.scalar_tensor_tensor(m2[:], i2[:], float(inv_a2), g2[:],
                                               ALU.mult, ALU.mult)
                S = pool.tile([NP, F], F16, tag="tmB", bufs=2, name=f"S_{k}")
                nc.vector.tensor_tensor(S[:], f2[:], st[k]["ch2"][:], ALU.mult)
                ch3 = pool.tile([NP, F], F16, tag="ch", bufs=3, name=f"ch3_{k}")
                nc.vector.tensor_tensor(ch3[:], S[:], m2[:], ALU.add)
                st[k]["ch3"] = ch3

            def TC3(k):
                tc3 = pool.tile([NP, F], F16, tag="tc", bufs=2, name=f"tc3_{k}")
                nc.scalar.activation(tc3[:], st[k]["ch3"][:], AF.Tanh,
                                     bias=0.0, scale=float(a2))
                st[k]["tc3"] = tc3

            def H3(k):
                h3 = pool.tile([NP, F], F16, tag="h3", bufs=2, name=f"h3_{k}")
                nc.vector.tensor_tensor(h3[:], st[k]["g2"][3][:], st[k]["tc3"][:], ALU.mult)
                st[k]["h3"] = h3

            def MM(k):
                pts = []
                for (c0, cw) in CHUNKS:
                    pt = psum_pool.tile([NQ, cw], dt.float32, tag="pt", bufs=8,
                                        name=f"pt_{k}_{c0}")
                    pts.append(pt)
                st[k]["pt"] = pts
                srcs = (st[k]["tau"], st[k]["h2"], st[k]["h3"])
                for ti in range(3):
                    src = srcs[ti]
                    for ci, (c0, cw) in enumerate(CHUNKS):
                        nc.tensor.matmul(pts[ci][:], wt[ti][:], src[:, c0:c0 + cw],
                                         start=(ti == 0), stop=(ti == 2))

            def CP(k, rng):
                outs = st[k].get("outs")
                if outs is None:
                    outs = pool.tile([NQ, F], F16, tag="outs", bufs=2, name=f"outs_{k}")
                    st[k]["outs"] = outs
                for ci in rng:
                    c0, cw = CHUNKS[ci]
                    nc.scalar.activation(outs[:, c0:c0 + cw], st[k]["pt"][ci][:],
                                         AF.Copy, bias=0.0, scale=1.0)

            def OUT(k):
                nc.sync.dma_start(outd[k], st[k]["outs"][:])

            # ---- software pipeline ----
            DMA(0)
            DMA(1)
            TAU(0)
            S1(0)
            TAU(1)
            for k in range(T):
                if k + 2 < T:
                    DMA(k + 2)
                if k + 1 < T:
                    S1(k + 1)
                G1(k)
                CH2(k)
                if k + 2 < T:
                    TAU(k + 2)
                TC2(k)
                if k >= 1:
                    CP(k - 1, (0, 1, 2))
                H2(k)
                S2(k)
                G2(k, (0, 1, 2))
                G2(k, (3,))
                CH3(k)
                if k >= 1:
                    CP(k - 1, (3,))
                TC3(k)
                H3(k)
                if k >= 1:
                    OUT(k - 1)
                MM(k)
            CP(T - 1, (0, 1, 2, 3))
            OUT(T - 1)

    nc.finalize()
    return nc


def kernel(x, w_ih, w_hh, b_ih, b_hh, w_lin, b_lin):
    from concourse.bass_utils import run_bass_kernel_spmd

    x = np.asarray(x, dtype=np.float32)
    w_ih = np.asarray(w_ih, dtype=np.float32)
    w_hh = np.asarray(w_hh, dtype=np.float32)
    b_ih = np.asarray(b_ih, dtype=np.float32)
    b_hh = np.asarray(b_hh, dtype=np.float32)
    w_lin = np.asarray(w_lin, dtype=np.float32)
    b_lin = np.asarray(b_lin, dtype=np.float32)

    wi, wf, wg, wo = (float(v) for v in w_ih[:, 0])
    ui, uf, ug, uo = (float(v) for v in w_hh[:, 0])
    bias = b_ih + b_hh
    bi, bf, bg, bo = (float(v) for v in bias)
    wl = w_lin[0].astype(np.float64)          # [18]
    bl = float(b_lin[0])

    wkey = (wi, wf, wg, wo, ui, uf, ug, uo, bi, bf, bg, bo)
    if wkey not in _CACHE:
        al, be, a, d, a2, d2, rms_h, rms_c = _fit_shared_tanh(wi, wf, wg, wo, bi, bg, bo)
        fit_ok = bool(rms_h < 5e-3 and rms_c < 8e-3 and abs(a) > 1e-3
                      and abs(a2) > 1e-3
                      and all(abs(u_ * a) > 1e-4 for u_ in (ui, uf, ug, uo)))
        if not fit_ok:
            al, be, a, d, a2, d2 = 0.0, 0.0, 1.0, 0.0, 1.0, 0.0
        key = wkey + (fit_ok, al, be, a, d, a2, d2)
        _CACHE[wkey] = (key, _build_kernel(key))
    key, nc = _CACHE[wkey]
    fit_ok, al, be, a, d, a2, d2 = key[12:]

    # Linear stationaries: W_t[p, q] = wl[3*(p%6) + t] if q == p//6
    # (t=0 operand is tau, so W_0 absorbs the h1 = a*tau + d scale; the +d
    #  offset becomes a constant added on host.)
    p = np.arange(NP)
    tscale = (a, 1.0, 1.0)
    wmats = []
    for t in range(3):
        W = np.zeros((NP, NQ), dtype=np.float16)
        W[p, p // 6] = (wl[3 * (p % 6) + t] * tscale[t]).astype(np.float16)
        wmats.append(W)
    bl_tot = np.float32(bl + d * wl[0::3].sum())

    # Host data prep: [B, 3, 1] -> per-core [T, NP, SEQ*F] fp16.
    xb = x.reshape(B, SEQ).astype(np.float16)
    in_maps = []
    for c in range(N_CORES):
        xc = xb[c * Bc:(c + 1) * Bc]
        xp = np.zeros((PAD_E, SEQ), dtype=np.float16)
        xp[:Bc] = xc
        # element e = ((tile*21 + q)*F + j)*6 + m  ->  [tile][q*6+m][t*F + j]
        xr = xp.reshape(T, NQ, F, 6, SEQ).transpose(0, 1, 3, 4, 2)
        xr = np.ascontiguousarray(xr).reshape(T, NP, SEQ * F)
        in_maps.append({"x": xr, "w1": wmats[0], "w2": wmats[1], "w3": wmats[2]})

    res = run_bass_kernel_spmd(nc, in_maps, list(range(N_CORES)))

    out = np.empty((B // 6, 1), dtype=np.float32)
    for c in range(N_CORES):
        oc = res.results[c]["out"].reshape(-1)[:GC].astype(np.float32)
        out[c * GC:(c + 1) * GC, 0] = oc + bl_tot
    return out
